# revision 10
# baseline (speedup 1.0000x reference)
"""Trainium2 Bass kernel for nn_Attention_2 (B=32, LQ=LK=2048, H=1024, A=512).

Math: the reference computes softmax over sum_q(Qp @ Kp^T), and the q-sum
distributes through the matmul, so the full [B, LQ, LK] score tensor never
needs to exist:

  qs[b]    = sum_q query[b,q,:]                       (query reduction)
  qp[b]    = qs[b] @ Wq + LQ*bq                       (tiny)
  wqt[b]   = Wk @ qp[b]         [H]                   (tiny)
  s[b,k]   = key[b,k,:] . wqt[b]   (+ const, cancels in softmax)
  vt[b,k]  = key[b,k,:] . wvt      (wvt = Wk @ Wv[:,0])
  x[b]     = sum_k softmax(s)[k] * vt[b,k] + (bk.Wv + bv)

Sharding: data-parallel over batch, 4 batches per core, 8 cores.
Per core the only heavy work is streaming query+key (67MB) from HBM:
  - query is summed over q via chained DMA-accumulate (free)
  - s/vt contractions over h are split between the PE (transpose + fp32
    matmul route, batches 0,2) and the DVE (fused mul+reduce route, 1,3)
    to keep both engines under the DMA roofline. Scores must be full fp32:
    the top-2 logit gap can be ~4, so bf16-level noise would flip ranks.
"""
import numpy as np

import concourse.bass as bass
import concourse.bacc as bacc
import concourse.tile as tile
from concourse import mybir
from concourse.bass_utils import run_bass_kernel_spmd

N_CORES = 8
B, LQ, LK, H, A = 32, 2048, 2048, 1024, 512
BPC = B // N_CORES          # batches per core
P = 128
f32 = mybir.dt.float32
NQT = LQ // P               # 16 q subtiles per batch
NKT = LK // P               # 16 k subtiles per batch
NG = 8                      # key granules per batch
GK = LK // NG               # 512 k rows per granule
HJ = H // P                 # 8 h-chunks
AC = A // P                 # 4 a-chunks

_CACHE = {}


def _emit_query_chain(nc, sbq, query, b):
    """Sum query[b] over the q axis into [128, H] q-partials: 4 chained
    2MB accumulate-DMAs, each adding 4 q-subtiles into 4 disjoint column
    blocks; two in-place DVE adds fold the 4 blocks into block 0."""
    qt = sbq.tile([P, 4 * H], f32, tag="qchain")
    for i in range(NQT // 4):
        kw = {} if i == 0 else {"accum_op": mybir.AluOpType.add}
        nc.gpsimd.dma_start(
            out=qt[:].rearrange("p (c h) -> p c h", c=4),
            in_=query[b, i * 4 * P:(i + 1) * 4 * P, :]
            .rearrange("(c p) h -> p c h", p=P),
            **kw)
    nc.vector.tensor_add(qt[:, 0:2 * H], qt[:, 0:2 * H], qt[:, 2 * H:4 * H])
    nc.vector.tensor_add(qt[:, 0:H], qt[:, 0:H], qt[:, H:2 * H])
    return qt[:, 0:H]


def build_bass(repeat=1):
    nc = bacc.Bacc(None, target_bir_lowering=False, debug=False)

    query = nc.dram_tensor("query", [BPC, LQ, H], f32, kind="ExternalInput").ap()
    key = nc.dram_tensor("key", [BPC, LK, H], f32, kind="ExternalInput").ap()
    Wq = nc.dram_tensor("Wq", [H, A], f32, kind="ExternalInput").ap()
    bq = nc.dram_tensor("bq", [A], f32, kind="ExternalInput").ap()
    Wk = nc.dram_tensor("Wk", [H, A], f32, kind="ExternalInput").ap()
    bk = nc.dram_tensor("bk", [A], f32, kind="ExternalInput").ap()
    Wv = nc.dram_tensor("Wv", [A, 1], f32, kind="ExternalInput").ap()
    bv = nc.dram_tensor("bv", [1], f32, kind="ExternalInput").ap()
    out = nc.dram_tensor("out", [BPC, 1], f32, kind="ExternalOutput").ap()

    with tile.TileContext(nc) as tc:
        for _ in range(repeat):
            _build_body(nc, tc, query, key, Wq, bq, Wk, bk, Wv, bv, out)
    nc.compile()
    return nc


def _build_body(nc, tc, query, key, Wq, bq, Wk, bk, Wv, bv, out):
    from contextlib import ExitStack
    ctx = ExitStack()
    with ctx:
        sbc = ctx.enter_context(tc.tile_pool(name="sbc", bufs=1))
        sbq = ctx.enter_context(tc.tile_pool(name="sbq", bufs=2))
        sbkey = ctx.enter_context(tc.tile_pool(name="sbkey", bufs=2))  # per-tag below
        sbkt = ctx.enter_context(tc.tile_pool(name="sbkt", bufs=3))
        sbrt = ctx.enter_context(tc.tile_pool(name="sbrt", bufs=2))
        sbsv = ctx.enter_context(tc.tile_pool(name="sbsv", bufs=1))
        sbsm = ctx.enter_context(tc.tile_pool(name="sbsm", bufs=2))
        sbw3 = ctx.enter_context(tc.tile_pool(name="sbw3", bufs=1))
        sbjunk = ctx.enter_context(tc.tile_pool(name="sbjunk", bufs=1))
        ps_keyT = ctx.enter_context(tc.tile_pool(name="ps_keyT", bufs=2, space="PSUM"))
        ps_s2 = ctx.enter_context(tc.tile_pool(name="ps_s2", bufs=2, space="PSUM"))
        ps_small = ctx.enter_context(tc.tile_pool(name="ps_small", bufs=1, space="PSUM"))

        # ---------------- constants ----------------
        ident = sbc.tile([P, P], f32)
        colidx = sbsm.tile([P, P], f32, tag="small")
        rowidx = sbsm.tile([P, 1], f32, tag="tiny")
        nc.gpsimd.iota(colidx[:], pattern=[[1, P]], base=0, channel_multiplier=0,
                       allow_small_or_imprecise_dtypes=True)
        nc.gpsimd.iota(rowidx[:], pattern=[[0, 1]], base=0, channel_multiplier=1,
                       allow_small_or_imprecise_dtypes=True)
        nc.vector.tensor_scalar(out=ident[:], in0=colidx[:], scalar1=rowidx[:],
                                scalar2=None, op0=mybir.AluOpType.is_equal)

        ones = sbc.tile([P, 1], f32)
        nc.vector.memset(ones[:], 1.0)
        ones_k1 = sbc.tile([1, P], f32)
        nc.vector.memset(ones_k1[:], 1.0)

        # Wq natural layout: [h-part, (j a)] ; chunk j at cols [j*A, (j+1)*A)
        Wq_sb = sbc.tile([P, HJ * A], f32)
        nc.sync.dma_start(out=Wq_sb[:].rearrange("p (j a) -> p j a", j=HJ),
                          in_=Wq.rearrange("(j p) a -> p j a", p=P))

        wv_sb = sbc.tile([P, AC], f32)
        nc.sync.dma_start(out=wv_sb[:].rearrange("p (c o) -> p c o", c=AC),
                          in_=Wv.rearrange("(c p) o -> p c o", p=P))
        bk_sb = sbc.tile([P, AC], f32)
        nc.sync.dma_start(out=bk_sb[:], in_=bk.rearrange("(c p) -> p c", p=P))
        bv_sb = sbc.tile([1, 1], f32)
        nc.sync.dma_start(out=bv_sb[:], in_=bv[None, :])
        bq2 = sbc.tile([2, A], f32)
        nc.sync.dma_start(out=bq2[:], in_=bass.AP(
            tensor=bq.tensor, offset=bq.offset, ap=[[0, 2]] + list(bq.ap)))

        # WkT [a-part, (c h)]: transpose Wk once on the PE (in two halves
        # to halve the SBUF staging footprint)
        WkT_sb = sbc.tile([P, AC * H], f32)
        with tc.tile_pool(name="sbwk", bufs=1) as sbwk:
            for half in range(2):
                Wk_sb = sbwk.tile([P, (HJ // 2) * A], f32, tag="wk")
                nc.sync.dma_start(
                    out=Wk_sb[:].rearrange("p (j a) -> p j a", j=HJ // 2),
                    in_=Wk[half * (H // 2):(half + 1) * (H // 2), :]
                    .rearrange("(j p) a -> p j a", p=P))
                for c in range(AC):
                    wkt_ps = ps_small.tile([P, (HJ // 2) * P], f32, tag="small")
                    for jl in range(HJ // 2):
                        nc.tensor.transpose(
                            wkt_ps[:, jl * P:(jl + 1) * P],
                            Wk_sb[:, jl * A + c * P: jl * A + (c + 1) * P],
                            ident[:])
                    dst = WkT_sb[:, c * H + half * (H // 2):
                                 c * H + (half + 1) * (H // 2)]
                    if c % 2 == 0:
                        nc.scalar.copy(dst, wkt_ps[:])
                    else:
                        nc.vector.tensor_copy(dst, wkt_ps[:])

        # c_v = bk . Wv + bv  (folded into the output at the very end)
        junk4 = sbsm.tile([P, AC], f32, tag="tiny2")
        cvcol = sbsm.tile([P, 1], f32, tag="tiny3")
        nc.vector.scalar_tensor_tensor(out=junk4[:], in0=bk_sb[:], scalar=1.0,
                                       in1=wv_sb[:], op0=mybir.AluOpType.mult,
                                       op1=mybir.AluOpType.mult, accum_out=cvcol[:])
        cv_ps = ps_small.tile([1, 1], f32, tag="small")
        nc.tensor.matmul(cv_ps[:], cvcol[:], ones[:], start=True, stop=True)
        cv_sb = sbc.tile([1, 1], f32)
        nc.vector.tensor_tensor(out=cv_sb[:], in0=cv_ps[:], in1=bv_sb[:],
                                op=mybir.AluOpType.add)

        # ---------------- per-pair processing ----------------
        for pair in range(2):
            b_pe, b_dve = 2 * pair, 2 * pair + 1

            # --- query sums (DMA-accumulate chains) ---
            qs_pe = _emit_query_chain(nc, sbq, query, b_pe)
            qs_dve = _emit_query_chain(nc, sbq, query, b_dve)

            # --- qsT columns: [h, 1] per (b, j) via ones-matmul ---
            qsT_ps = ps_small.tile([P, 2 * HJ], f32, tag="small")
            for slot, qs in ((0, qs_pe), (1, qs_dve)):
                for j in range(HJ):
                    nc.tensor.matmul(qsT_ps[:, 2 * j + slot: 2 * j + slot + 1],
                                     qs[:, j * P:(j + 1) * P], ones[:],
                                     start=True, stop=True)
            qsT_sb = sbsm.tile([P, 2 * HJ], f32, tag="qsT")
            nc.vector.tensor_copy(qsT_sb[:], qsT_ps[:])

            # --- qp = qs @ Wq + LQ*bq   [2, A] (rows = pair slots) ---
            qp_ps = ps_small.tile([2, A], f32, tag="small")
            for j in range(HJ):
                nc.tensor.matmul(qp_ps[:], qsT_sb[:, 2 * j:2 * j + 2],
                                 Wq_sb[:, j * A:(j + 1) * A],
                                 start=(j == 0), stop=(j == HJ - 1))
            qp_sb = sbsm.tile([2, A], f32, tag="qp")
            nc.scalar.copy(qp_sb[:], qp_ps[:])
            qp2 = sbsm.tile([2, A], f32, tag="qp2")
            nc.vector.scalar_tensor_tensor(out=qp2[:], in0=bq2[:], scalar=float(LQ),
                                           in1=qp_sb[:], op0=mybir.AluOpType.mult,
                                           op1=mybir.AluOpType.add)
            qp2sw = sbsm.tile([2, A], f32, tag="qp2sw")
            nc.vector.stream_shuffle(qp2sw[:], qp2[:], [1, 0] + list(range(2, 32)))

            # --- w3 = [qpT_pe | qpT_dve | wvT] in a-partition layout ---
            w3_ps = ps_small.tile([P, 3 * AC], f32, tag="small")
            for c in range(AC):
                nc.tensor.transpose(w3_ps[:, 3 * c: 3 * c + 1],
                                    qp2[0:1, c * P:(c + 1) * P], ident[0:1, 0:1])
                nc.tensor.transpose(w3_ps[:, 3 * c + 1: 3 * c + 2],
                                    qp2sw[0:1, c * P:(c + 1) * P], ident[0:1, 0:1])
            w3_sb = sbsm.tile([P, 3 * AC], f32, tag="w3")
            nc.vector.tensor_copy(w3_sb[:], w3_ps[:])
            nc.vector.tensor_copy(
                w3_sb[:].rearrange("p (c s) -> p c s", c=AC)[:, :, 2:3],
                wv_sb[:].rearrange("p (c o) -> p c o", c=AC))

            # --- [wqt_pe; wqt_dve; wvt] = w3^T @ WkT   -> [3, H] ---
            w3o_ps = ps_small.tile([3, H], f32, tag="small")
            for half in range(2):
                for c in range(AC):
                    nc.tensor.matmul(
                        w3o_ps[:, half * A:(half + 1) * A],
                        w3_sb[:, 3 * c: 3 * c + 3],
                        WkT_sb[:, c * H + half * A: c * H + (half + 1) * A],
                        start=(c == 0), stop=(c == AC - 1))
            w3o = sbw3.tile([3, H], f32, tag="w3o")
            nc.scalar.copy(w3o[:], w3o_ps[:])
            w3o_r1 = sbw3.tile([3, H], f32, tag="w3or1")
            nc.vector.stream_shuffle(w3o_r1[:], w3o[:], [1, 2, 0] + list(range(3, 32)))
            w3o_r2 = sbw3.tile([3, H], f32, tag="w3or2")
            nc.vector.stream_shuffle(w3o_r2[:], w3o[:], [2, 0, 1] + list(range(3, 32)))

            # --- PE-route prep: w2 [h-part, (j 2)] = [wqt_chunk, wvt_chunk] ---
            w2ps = ps_small.tile([P, 2 * HJ], f32, tag="small")
            for j in range(HJ):
                nc.tensor.transpose(w2ps[:, 2 * j: 2 * j + 1],
                                    w3o[0:1, j * P:(j + 1) * P], ident[0:1, 0:1])
                nc.tensor.transpose(w2ps[:, 2 * j + 1: 2 * j + 2],
                                    w3o_r2[0:1, j * P:(j + 1) * P], ident[0:1, 0:1])
            w2 = sbrt.tile([P, 2 * HJ], f32, tag="w2")
            nc.vector.tensor_copy(w2[:], w2ps[:])

            # --- DVE-route prep: broadcast wqt_dve and wvt across partitions ---
            wqbc = sbrt.tile([P, H], f32, tag="wqbc")
            bc_ps = ps_small.tile([P, H], f32, tag="small")
            for half in range(2):
                nc.tensor.matmul(bc_ps[:, half * A:(half + 1) * A], ones_k1[:],
                                 w3o_r1[0:1, half * A:(half + 1) * A],
                                 start=True, stop=True)
            nc.scalar.copy(wqbc[:], bc_ps[:])
            wvbc = sbrt.tile([P, H], f32, tag="wvbc")
            bc_ps2 = ps_small.tile([P, H], f32, tag="small")
            for half in range(2):
                nc.tensor.matmul(bc_ps2[:, half * A:(half + 1) * A], ones_k1[:],
                                 w3o_r2[0:1, half * A:(half + 1) * A],
                                 start=True, stop=True)
            nc.scalar.copy(wvbc[:], bc_ps2[:])

            # --- key pass ---
            sv_sb = sbsv.tile([2, LK], f32, tag="sv")          # PE-route scores/v rows
            sdve = sbsv.tile([P, NKT], f32, tag="sdve")        # DVE-route scores
            vdve = sbsv.tile([P, NKT], f32, tag="vdve")

            for g in range(NG):
                kt_pe = sbkey.tile([P, (GK // P) * H], f32, tag="ktpe", bufs=3)
                nc.sync.dma_start(out=kt_pe[:].rearrange("p (n h) -> p n h", n=GK // P),
                                  in_=key[b_pe, g * GK:(g + 1) * GK, :]
                                  .rearrange("(n p) h -> p n h", p=P))
                kt_dve = sbkey.tile([P, (GK // P) * H], f32, tag="ktdve")
                nc.sync.dma_start(out=kt_dve[:].rearrange("p (n h) -> p n h", n=GK // P),
                                  in_=key[b_dve, g * GK:(g + 1) * GK, :]
                                  .rearrange("(n p) h -> p n h", p=P))
                for n in range(GK // P):
                    t = g * (GK // P) + n
                    kv = kt_pe[:, n * H:(n + 1) * H]
                    # PE route: transpose then fp32 matmul with [wqt|wvt]
                    keyT_ps = ps_keyT.tile([P, H], f32, tag="keyT")
                    for j in range(HJ):
                        nc.tensor.transpose(keyT_ps[:, j * P:(j + 1) * P],
                                            kv[:, j * P:(j + 1) * P], ident[:])
                    keyT = sbkt.tile([P, H], f32, tag="keyT")
                    nc.scalar.copy(keyT[:], keyT_ps[:])
                    s2_ps = ps_s2.tile([2, P], f32, tag="s2")
                    for j in range(HJ):
                        nc.tensor.matmul(s2_ps[:], w2[:, 2 * j:2 * j + 2],
                                         keyT[:, j * P:(j + 1) * P],
                                         start=(j == 0), stop=(j == HJ - 1))
                    nc.scalar.copy(sv_sb[:, t * P:(t + 1) * P], s2_ps[:])

                    # DVE route: fused mul+reduce rowwise dots
                    kvd = kt_dve[:, n * H:(n + 1) * H]
                    junk = sbjunk.tile([P, H], f32, tag="junk")
                    nc.vector.scalar_tensor_tensor(
                        out=junk[:], in0=kvd[:], scalar=1.0, in1=wqbc[:],
                        op0=mybir.AluOpType.mult, op1=mybir.AluOpType.mult,
                        accum_out=sdve[:, t:t + 1])
                    junk2 = sbjunk.tile([P, H], f32, tag="junk")
                    nc.vector.scalar_tensor_tensor(
                        out=junk2[:], in0=kvd[:], scalar=1.0, in1=wvbc[:],
                        op0=mybir.AluOpType.mult, op1=mybir.AluOpType.mult,
                        accum_out=vdve[:, t:t + 1])

            # --- softmax + combine, PE-route batch ---
            vsw = sbjunk.tile([2, LK], f32, tag="vsw")
            nc.vector.stream_shuffle(vsw[:], sv_sb[:], [1, 0] + list(range(2, 32)))
            smax = sbsm.tile([2, 1], f32, tag="smax")
            nc.vector.reduce_max(smax[:], sv_sb[:], axis=mybir.AxisListType.X)
            nmax = sbsm.tile([2, 1], f32, tag="nmax")
            nc.vector.tensor_scalar_mul(nmax[:], smax[:], -1.0)
            e_row = sbjunk.tile([1, LK], f32, tag="erow")
            den = sbsm.tile([1, 1], f32, tag="den")
            nc.scalar.activation(e_row[:], sv_sb[0:1, :],
                                 mybir.ActivationFunctionType.Exp,
                                 bias=nmax[0:1], scale=1.0, accum_out=den[:])
            junk3 = sbjunk.tile([1, LK], f32, tag="junk")
            num = sbsm.tile([1, 1], f32, tag="num")
            nc.vector.scalar_tensor_tensor(
                out=junk3[:], in0=e_row[:], scalar=1.0, in1=vsw[0:1, :],
                op0=mybir.AluOpType.mult, op1=mybir.AluOpType.mult,
                accum_out=num[:])
            _emit_final(nc, sbsm, num, den, cv_sb, out, b_pe)

            # --- softmax + combine, DVE-route batch ---
            m1 = sbsm.tile([P, 1], f32, tag="m1")
            nc.vector.reduce_max(m1[:], sdve[:], axis=mybir.AxisListType.X)
            mT_ps = ps_small.tile([1, P], f32, tag="small")
            nc.tensor.transpose(mT_ps[:], m1[:], ident[:])
            mT_sb = sbsm.tile([1, P], f32, tag="mT")
            nc.vector.tensor_copy(mT_sb[:], mT_ps[:])
            gmax = sbsm.tile([1, 1], f32, tag="gmax")
            nc.vector.reduce_max(gmax[:], mT_sb[:], axis=mybir.AxisListType.X)
            ng_ps = ps_small.tile([P, 1], f32, tag="small")
            nc.tensor.matmul(ng_ps[:], ones_k1[:], gmax[:], start=True, stop=True)
            ngm = sbsm.tile([P, 1], f32, tag="ngm")
            nc.vector.tensor_scalar_mul(ngm[:], ng_ps[:], -1.0)
            e128 = sbsm.tile([P, NKT], f32, tag="e128")
            erow = sbsm.tile([P, 1], f32, tag="erowp")
            nc.scalar.activation(e128[:], sdve[:], mybir.ActivationFunctionType.Exp,
                                 bias=ngm[:], scale=1.0, accum_out=erow[:])
            junk5 = sbsm.tile([P, NKT], f32, tag="junk5")
            nrow = sbsm.tile([P, 1], f32, tag="nrow")
            nc.vector.scalar_tensor_tensor(
                out=junk5[:], in0=e128[:], scalar=1.0, in1=vdve[:],
                op0=mybir.AluOpType.mult, op1=mybir.AluOpType.mult,
                accum_out=nrow[:])
            den_ps = ps_small.tile([1, 2], f32, tag="small")
            nc.tensor.matmul(den_ps[:, 0:1], erow[:], ones[:], start=True, stop=True)
            nc.tensor.matmul(den_ps[:, 1:2], nrow[:], ones[:], start=True, stop=True)
            dn = sbsm.tile([1, 2], f32, tag="dn")
            nc.vector.tensor_copy(dn[:], den_ps[:])
            _emit_final(nc, sbsm, dn[:, 1:2], dn[:, 0:1], cv_sb, out, b_dve)


def _emit_final(nc, sbsm, num, den, cv_sb, out, b):
    rden = sbsm.tile([1, 1], f32, tag="rden")
    nc.vector.reciprocal(rden[:], den[:])
    x = sbsm.tile([1, 1], f32, tag="x")
    nc.vector.tensor_tensor(out=x[:], in0=num[:], in1=rden[:],
                            op=mybir.AluOpType.mult)
    x2 = sbsm.tile([1, 1], f32, tag="x2")
    nc.vector.tensor_tensor(out=x2[:], in0=x[:], in1=cv_sb[:],
                            op=mybir.AluOpType.add)
    nc.sync.dma_start(out=out[b:b + 1, :], in_=x2[:])


def _shard(query, key, shared):
    in_maps = []
    for c in range(N_CORES):
        sl = slice(c * BPC, (c + 1) * BPC)
        m = {"query": np.ascontiguousarray(query[sl]),
             "key": np.ascontiguousarray(key[sl])}
        m.update(shared)
        in_maps.append(m)
    return in_maps


def _make_in_maps(inputs):
    query = np.ascontiguousarray(np.asarray(inputs["query"], dtype=np.float32))
    key = np.ascontiguousarray(np.asarray(inputs["key"], dtype=np.float32))
    shared = {k: np.ascontiguousarray(np.asarray(inputs[k], dtype=np.float32))
              for k in ("Wq", "bq", "Wk", "bk", "Wv", "bv")}
    return _shard(query, key, shared)


def kernel(**inputs):
    if "nc" not in _CACHE:
        _CACHE["nc"] = build_bass()
    nc = _CACHE["nc"]

    query = np.ascontiguousarray(np.asarray(inputs["query"], dtype=np.float32))
    key = np.ascontiguousarray(np.asarray(inputs["key"], dtype=np.float32))
    shared = {
        "Wq": np.ascontiguousarray(np.asarray(inputs["Wq"], dtype=np.float32)),
        "bq": np.ascontiguousarray(np.asarray(inputs["bq"], dtype=np.float32)),
        "Wk": np.ascontiguousarray(np.asarray(inputs["Wk"], dtype=np.float32)),
        "bk": np.ascontiguousarray(np.asarray(inputs["bk"], dtype=np.float32)),
        "Wv": np.ascontiguousarray(np.asarray(inputs["Wv"], dtype=np.float32)),
        "bv": np.ascontiguousarray(np.asarray(inputs["bv"], dtype=np.float32)),
    }
    in_maps = _shard(query, key, shared)

    res = run_bass_kernel_spmd(nc, in_maps, list(range(N_CORES)))
    outs = [res.results[c]["out"] for c in range(N_CORES)]
    return np.concatenate(outs, axis=0).astype(np.float32)


if __name__ == "__main__":
    rng = np.random.default_rng(0)
    ins = {
        "query": rng.standard_normal((B, LQ, H), dtype=np.float32),
        "key": rng.standard_normal((B, LK, H), dtype=np.float32),
        "Wq": (rng.standard_normal((H, A), dtype=np.float32) / np.sqrt(H)).astype(np.float32),
        "bq": np.zeros((A,), np.float32),
        "Wk": (rng.standard_normal((H, A), dtype=np.float32) / np.sqrt(H)).astype(np.float32),
        "bk": np.zeros((A,), np.float32),
        "Wv": (rng.standard_normal((A, 1), dtype=np.float32) / np.sqrt(A)).astype(np.float32),
        "bv": np.zeros((1,), np.float32),
    }
    x = kernel(**ins)
    print("kernel out:", x[:8, 0])
